# revision 5
# baseline (speedup 1.0000x reference)
"""Trainium2 Bass kernel for nn_GAttention (gnn_message_passing).

Reference computation (per batch b):
    q = s[:,b,:] @ Qweight                      # (N, H)
    k = Kweight.T @ s[:,b,:]                    # (H, I)   (contraction over n)
    att1 = (q @ k) * (1/sqrt(H)) + 1e-9         # (N, I)
    att2 = att1**2 @ Gmat                       # (N, I)
    out[:,b,:] = att2 / (rowsum(att2) + 1e-3)

Sharding: pure data-parallel over batch B=16 -> 2 batches per core on 8 cores.
Gmat/Qweight/Kweight replicated.

Measured bottlenecks this kernel is built around (from ntff profiles):
  * DMA is issue-rate bound: ~8ns/packet on one HWDGE ring regardless of
    packet size, so every tensor is HOST-PACKED into its exact SBUF tile
    layout and moved with few, large, fully-contiguous dma_starts
    (>=512KB each, ~340GB/s instead of ~170GB/s chunked).
  * The PE streams ~1 moving column/cycle regardless of dtype; fp8
    MatmulPerfMode.DoubleRow doubles K per column (measured 2x vs bf16).
    So s ships in fp8 e4m3 in BOTH layouts (s_n for k, s_T for q) and
    k/q/att2 all run DoubleRow fp8. att1 (K=64 only) stays bf16.
  * Qweight/Kweight are pre-scaled by 32 on the host so their values sit
    in fp8's normal range; the compensating 2^-13 (= (1/8)/32/32) is
    folded into the att1 PSUM eviction (exact power-of-two scales).
    The +1e-9 inside the square is dropped (relative ~3e-9).
  * Vector-engine work is spread over three engines: ACT evicts att2
    PSUM->f16 with fused accum_out rowsums, DVE evicts att1 PSUM->bf16
    (scaled) and builds 1/(rs0+rs1+1e-3), GPSIMD (SBUF-only engine)
    squares att1 (bf16->fp8) and applies the row normalization in place.
  * Outputs stage per-batch in SBUF f16 and leave in 1MB DMAs on the
    second HWDGE ring (nc.scalar) so they never block input streaming.

The two batches are software-pipelined: batch 1's k/q/att1 phases are
woven into batch 0's att2 group stream so the PE never idles long enough
for the HAM clock gate to throttle.
"""

import sys

import numpy as np

try:  # concourse normally comes from the image's NIX_PYTHONPATH
    import concourse  # noqa: F401
except ImportError:  # pragma: no cover
    sys.path.insert(0, "/opt/trn_rl_repo")

N_DIM = 1024
IN_DIM = 1024
H_DIM = 64
B = 16
N_CORES = 8
B_LOC = B // N_CORES  # batches per core

P = 128          # SBUF/PSUM partitions
NCH = 8          # chunks over n / i (1024/128)
NH = 512         # psum free-dim half (one fp32 bank)

# "A": s/q/k bf16 (kq plain matmuls), att2 fp8 DoubleRow.
# "B": s/Qw/Kw fp8 -> kq matmuls DoubleRow too (half the PE columns + bytes).
MODE = "B"

_NC_CACHE = {}


def _build_nc(mode=MODE):
    import concourse.bass as bass  # noqa: F401
    import concourse.tile as tile
    from concourse import bacc, mybir

    f32 = mybir.dt.float32
    bf16 = mybir.dt.bfloat16
    f16 = mybir.dt.float16
    f8 = mybir.dt.float8e4
    AFT = mybir.ActivationFunctionType
    DR = mybir.MatmulPerfMode.DoubleRow
    ALU = mybir.AluOpType

    s_dt = f8 if mode == "B" else bf16
    w_dt = f8 if mode == "B" else bf16
    # mode B: Qw,Kw pre-scaled by 32 on host (fp8 denormal floor); att1
    # eviction multiplies by 2^-13 = 0.125/(32*32). mode A folds 0.125
    # into Qw directly, eviction scale 1.
    sq_scale = 2.0 ** -13 if mode == "B" else 1.0

    nc = bacc.Bacc(
        "TRN2",
        target_bir_lowering=False,
        debug=False,
        num_devices=N_CORES,
    )
    # All dram tensors are host-packed to match the SBUF tile layout, so
    # every DMA is a fully-contiguous per-partition transfer.
    sn_d = nc.dram_tensor("sn", [P, B_LOC, NCH, IN_DIM], s_dt, kind="ExternalInput")
    st_d = nc.dram_tensor("st", [P, B_LOC, NCH, N_DIM], s_dt, kind="ExternalInput")
    g_d = nc.dram_tensor("gmat", [P, NCH, IN_DIM], f8, kind="ExternalInput")
    qw_d = nc.dram_tensor("qw", [P, NCH, H_DIM], w_dt, kind="ExternalInput")
    kw_d = nc.dram_tensor("kw", [P, NCH, H_DIM], w_dt, kind="ExternalInput")
    o_d = nc.dram_tensor("out", [P, B_LOC, NCH, IN_DIM], f16, kind="ExternalOutput")

    with tile.TileContext(nc) as tc:
        with (
            tc.tile_pool(name="const", bufs=1) as const_pool,
            tc.tile_pool(name="gmat", bufs=1) as gmat_pool,
            tc.tile_pool(name="sn", bufs=2) as sn_pool,
            tc.tile_pool(name="st", bufs=2) as st_pool,
            tc.tile_pool(name="att1e", bufs=2) as att1e_pool,
            tc.tile_pool(name="att1q", bufs=2) as att1q_pool,
            tc.tile_pool(name="kq", bufs=2) as kq_pool,
            tc.tile_pool(name="outs", bufs=2) as out_pool,
            tc.tile_pool(name="stat", bufs=8) as stat_pool,
            tc.tile_pool(name="ps5", bufs=6, space="PSUM") as ps5,
            tc.tile_pool(name="psKQ", bufs=1, space="PSUM") as psKQ,
        ):
            qw_sb = const_pool.tile([P, NCH, H_DIM], w_dt)
            nc.sync.dma_start(qw_sb[:], qw_d.ap())
            kw_sb = const_pool.tile([P, NCH, H_DIM], w_dt)
            nc.sync.dma_start(kw_sb[:], kw_d.ap())

            g_sb = gmat_pool.tile([P, NCH, IN_DIM], f8)

            def phase_load_s(b):
                """Both layouts of s_b in 2 half-DMAs each (chunks 0-3, 4-7)."""
                sn_t = sn_pool.tile([P, NCH, IN_DIM], s_dt, tag="sn")
                st_t = st_pool.tile([P, NCH, N_DIM], s_dt, tag="st")
                halves = []
                for lo in (0, 4):
                    halves.append(nc.sync.dma_start(
                        sn_t[:, lo:lo + 4, :], sn_d.ap()[:, b, lo:lo + 4, :]
                    ))
                    halves.append(nc.sync.dma_start(
                        st_t[:, lo:lo + 4, :], st_d.ap()[:, b, lo:lo + 4, :]
                    ))
                return sn_t, st_t, halves

            def kq_matmuls(w_sb, s_t, ps):
                """PE instructions of one k or q chain (accum over chunks)."""
                ins = []
                if mode == "A":
                    for c in range(NCH):
                        for half in range(2):
                            ins.append(lambda c=c, half=half: nc.tensor.matmul(
                                ps[:, half * NH:(half + 1) * NH],
                                w_sb[:, c, :],
                                s_t[:, c, half * NH:(half + 1) * NH],
                                start=(c == 0),
                                stop=(c == NCH - 1),
                            ))
                else:
                    for ks in range(NCH // 2):
                        for half in range(2):
                            ins.append(lambda ks=ks, half=half: nc.tensor.matmul(
                                ps[:, half * NH:(half + 1) * NH],
                                w_sb[:, 2 * ks:2 * ks + 2, :],
                                s_t[:, 2 * ks:2 * ks + 2, half * NH:(half + 1) * NH],
                                start=(ks == 0),
                                stop=(ks == NCH // 2 - 1),
                                perf_mode=DR,
                            ))
                return ins

            def emit_evict_kq(ps, tag):
                sb = kq_pool.tile([H_DIM, N_DIM], bf16, tag=tag)
                nc.vector.tensor_copy(sb[:], ps[:])
                return sb

            def emit_att1_group(att1e, att1sq, k_sb, q_sb, ci, half):
                """att1T tile (ci,half): matmul; DVE evict*2^-13 -> bf16;
                GPSIMD square -> fp8."""
                pa = ps5.tile([P, NH], f32, tag="ps512")
                nc.tensor.matmul(
                    pa[:],
                    k_sb[:, ci * P:(ci + 1) * P],
                    q_sb[:, half * NH:(half + 1) * NH],
                    start=True,
                    stop=True,
                )
                stage = att1e[:, ci, half * NH:(half + 1) * NH]
                nc.vector.tensor_scalar(
                    stage, pa[:], sq_scale, 0.0, op0=ALU.mult, op1=ALU.add,
                )
                nc.gpsimd.tensor_mul(
                    att1sq[:, ci, half * NH:(half + 1) * NH], stage, stage,
                )

            def phase_att2_group(b, att1sq, out_sb, nt):
                """One att2 row-block: DoubleRow matmuls, ACT evict+rowsums,
                DVE 1/(rs+1e-3), GPSIMD normalize in place."""
                po0 = ps5.tile([P, NH], f32, tag="ps512")
                po1 = ps5.tile([P, NH], f32, tag="ps512")
                for ks in range(NCH // 2):
                    lhsT = att1sq[:, 2 * ks:2 * ks + 2, nt * P:(nt + 1) * P]
                    nc.tensor.matmul(
                        po0[:], lhsT, g_sb[:, 2 * ks:2 * ks + 2, 0:NH],
                        start=(ks == 0), stop=(ks == NCH // 2 - 1),
                        perf_mode=DR,
                    )
                    nc.tensor.matmul(
                        po1[:], lhsT, g_sb[:, 2 * ks:2 * ks + 2, NH:2 * NH],
                        start=(ks == 0), stop=(ks == NCH // 2 - 1),
                        perf_mode=DR,
                    )
                dst = out_sb[:, nt, :]
                rs0 = stat_pool.tile([P, 1], f32, tag="rs0")
                rs1 = stat_pool.tile([P, 1], f32, tag="rs1")
                nc.scalar.activation(dst[:, 0:NH], po0[:], AFT.Copy, accum_out=rs0[:])
                nc.scalar.activation(dst[:, NH:2 * NH], po1[:], AFT.Copy, accum_out=rs1[:])
                rinv = stat_pool.tile([P, 1], f32, tag="rinv")
                nc.vector.scalar_tensor_tensor(
                    rinv[:], rs0[:], 1e-3, rs1[:], op0=ALU.add, op1=ALU.add,
                )
                nc.vector.reciprocal(rinv[:], rinv[:])
                nc.gpsimd.tensor_scalar_mul(dst, dst, rinv[:])

            def emit_out_dma(b, out_sb, lo):
                # second HWDGE ring (ACT-issued) so outputs don't block inputs
                nc.scalar.dma_start(
                    o_d.ap()[:, b, lo:lo + 4, :], out_sb[:, lo:lo + 4, :]
                )

            ATT1_ORDER = [(ci, half) for half in range(2) for ci in range(NCH)]

            # ---- DMA schedule (one FIFO ring): s(b0) -> G -> s(b1)
            sn0, st0, _ = phase_load_s(0)
            nc.sync.dma_start(g_sb[:], g_d.ap())
            sn1, st1, _ = phase_load_s(1)

            # ---- batch 0: k, q, att1
            ps_k0 = psKQ.tile([H_DIM, N_DIM], f32, tag="kq")
            for m in kq_matmuls(kw_sb, sn0, ps_k0):
                m()
            k_sb0 = emit_evict_kq(ps_k0, "k")
            ps_q0 = psKQ.tile([H_DIM, N_DIM], f32, tag="kq")
            for m in kq_matmuls(qw_sb, st0, ps_q0):
                m()
            q_sb0 = emit_evict_kq(ps_q0, "q")
            att1e0 = att1e_pool.tile([P, NCH, N_DIM], bf16, tag="att1e")
            att1sq0 = att1q_pool.tile([P, NCH, N_DIM], f8, tag="att1q")
            for ci, half in ATT1_ORDER:
                emit_att1_group(att1e0, att1sq0, k_sb0, q_sb0, ci, half)

            # ---- att2(b0) with k/q/att1 of batch 1 woven into the stream
            out_sb0 = out_pool.tile([P, NCH, IN_DIM], f16, tag="out")
            ps_k1 = psKQ.tile([H_DIM, N_DIM], f32, tag="kq")
            k_ins = kq_matmuls(kw_sb, sn1, ps_k1)
            nk = len(k_ins)
            att1e1 = att1e_pool.tile([P, NCH, N_DIM], bf16, tag="att1e")
            att1sq1 = att1q_pool.tile([P, NCH, N_DIM], f8, tag="att1q")
            holder = {}

            def weave_q_start():
                ps_q1 = psKQ.tile([H_DIM, N_DIM], f32, tag="kq")
                holder["q_ins"] = kq_matmuls(qw_sb, st1, ps_q1)
                holder["ps_q1"] = ps_q1

            for nt in range(NCH):
                phase_att2_group(0, att1sq0, out_sb0, nt)
                if nt == 3:
                    emit_out_dma(0, out_sb0, 0)
                if nt == 1:
                    for m in k_ins[0:nk // 2]:
                        m()
                elif nt == 2:
                    for m in k_ins[nk // 2:]:
                        m()
                    holder["k_sb1"] = emit_evict_kq(ps_k1, "k")
                    weave_q_start()
                elif nt == 3:
                    for m in holder["q_ins"][0:nk // 2]:
                        m()
                elif nt == 4:
                    for m in holder["q_ins"][nk // 2:]:
                        m()
                    holder["q_sb1"] = emit_evict_kq(holder["ps_q1"], "q")
                elif nt == 5:
                    for ci, half in ATT1_ORDER[0:5]:
                        emit_att1_group(att1e1, att1sq1,
                                        holder["k_sb1"], holder["q_sb1"], ci, half)
                elif nt == 6:
                    for ci, half in ATT1_ORDER[5:10]:
                        emit_att1_group(att1e1, att1sq1,
                                        holder["k_sb1"], holder["q_sb1"], ci, half)
                elif nt == 7:
                    for ci, half in ATT1_ORDER[10:16]:
                        emit_att1_group(att1e1, att1sq1,
                                        holder["k_sb1"], holder["q_sb1"], ci, half)
            emit_out_dma(0, out_sb0, 4)

            out_sb1 = out_pool.tile([P, NCH, IN_DIM], f16, tag="out")
            for nt in range(NCH):
                phase_att2_group(1, att1sq1, out_sb1, nt)
                if nt == 3:
                    emit_out_dma(1, out_sb1, 0)
            emit_out_dma(1, out_sb1, 4)

    nc.compile()
    return nc


def _get_nc(mode=MODE):
    if mode not in _NC_CACHE:
        _NC_CACHE[mode] = _build_nc(mode)
    return _NC_CACHE[mode]


def _pack(a):
    """[C*P, ...tail] -> [P, ...tail-with-C-inserted]: rows c*P+p -> [p][c]."""
    c = a.shape[0] // P
    return np.ascontiguousarray(
        a.reshape(c, P, *a.shape[1:]).swapaxes(0, 1)
    )


def _run(inputs, trace=False, mm_mode=None, tmpdir=None, mode=MODE):
    import ml_dtypes
    from concourse.bass_utils import run_bass_kernel_spmd

    bf16 = ml_dtypes.bfloat16
    f8 = ml_dtypes.float8_e4m3
    s_np = f8 if mode == "B" else bf16
    w_np = f8 if mode == "B" else bf16
    q_mult = 32.0 if mode == "B" else 0.125
    k_mult = 32.0 if mode == "B" else 1.0

    s = np.asarray(inputs["s"], dtype=np.float32)
    # pack to [P, NCH, IN_DIM]: row (c*128+p) -> [p, c]
    g8 = _pack(np.asarray(inputs["Gmat"], np.float32).astype(f8))
    qw = _pack((np.asarray(inputs["Qweight"], np.float32) * q_mult).astype(w_np))
    kw = _pack((np.asarray(inputs["Kweight"], np.float32) * k_mult).astype(w_np))

    s_c = s.astype(s_np)  # cast once, slice per core

    nc = _get_nc(mode)
    in_maps = []
    for c in range(N_CORES):
        sl = s_c[:, c * B_LOC:(c + 1) * B_LOC, :]        # [N, B_LOC, I]
        # sn[p, b, ch, i] = sl[ch*128+p, b, i]
        sn = _pack(sl).swapaxes(1, 2)                     # [P, B_LOC, NCH, I]
        # st[p, b, ch, n] = sl[n, b, ch*128+p]
        st = _pack(np.ascontiguousarray(sl.transpose(2, 1, 0))).swapaxes(1, 2)
        in_maps.append({
            "sn": np.ascontiguousarray(sn),
            "st": np.ascontiguousarray(st),
            "gmat": g8,
            "qw": qw,
            "kw": kw,
        })
    res = run_bass_kernel_spmd(
        nc, in_maps, list(range(N_CORES)), trace=trace, tmpdir=tmpdir
    )
    outs = []
    for c in range(N_CORES):
        o = res.results[c]["out"]                         # [P, B_LOC, NCH, I] f16
        # out[ch*128+p, b, j] = o[p, b, ch, j]
        outs.append(o.transpose(2, 0, 1, 3).reshape(N_DIM, B_LOC, IN_DIM))
    out = np.concatenate(outs, axis=1).astype(np.float32)
    return out, res


def kernel(**inputs) -> np.ndarray:
    out, _ = _run(inputs, trace=False)
    return out


# revision 7
# speedup vs baseline: 3.5129x; 3.5129x over previous
"""Trainium2 Bass kernel for nn_GAttention (gnn_message_passing).

Reference computation (per batch b):
    q = s[:,b,:] @ Qweight                      # (N, H)
    k = Kweight.T @ s[:,b,:]                    # (H, I)   (contraction over n)
    att1 = (q @ k) * (1/sqrt(H)) + 1e-9         # (N, I)
    att2 = att1**2 @ Gmat                       # (N, I)
    out[:,b,:] = att2 / (rowsum(att2) + 1e-3)

Sharding: pure data-parallel over batch B=16 -> 2 batches per core on 8 cores.
Gmat/Qweight/Kweight replicated.

Measured bottlenecks this kernel is built around (from ntff profiles):
  * DMA is issue-rate bound: ~8ns/packet on one HWDGE ring regardless of
    packet size, so every tensor is HOST-PACKED into its exact SBUF tile
    layout and moved with few, large, fully-contiguous dma_starts
    (>=512KB each, ~340GB/s instead of ~170GB/s chunked).
  * The PE streams ~1 moving column/cycle regardless of dtype; fp8
    MatmulPerfMode.DoubleRow doubles K per column (measured 2x vs bf16).
    So s ships in fp8 e4m3 in BOTH layouts (s_n for k, s_T for q) and
    k/q/att2 all run DoubleRow fp8. att1 (K=64 only) stays bf16.
  * Qweight/Kweight are pre-scaled by 32 on the host so their values sit
    in fp8's normal range; the compensating 2^-13 (= (1/8)/32/32) is
    folded into the att1 PSUM eviction (exact power-of-two scales).
    The +1e-9 inside the square is dropped (relative ~3e-9).
  * Vector-engine work is spread over three engines: ACT evicts att2
    PSUM->f16 with fused accum_out rowsums, DVE evicts att1 PSUM->bf16
    (scaled) and builds 1/(rs0+rs1+1e-3), GPSIMD (SBUF-only engine)
    squares att1 (bf16->fp8) and applies the row normalization in place.
  * Outputs stage per-batch in SBUF f16 and leave in 1MB DMAs on the
    second HWDGE ring (nc.scalar) so they never block input streaming.

The two batches are software-pipelined: batch 1's k/q/att1 phases are
woven into batch 0's att2 group stream so the PE never idles long enough
for the HAM clock gate to throttle.
"""

import sys

import numpy as np

try:  # concourse normally comes from the image's NIX_PYTHONPATH
    import concourse  # noqa: F401
except ImportError:  # pragma: no cover
    sys.path.insert(0, "/opt/trn_rl_repo")

N_DIM = 1024
IN_DIM = 1024
H_DIM = 64
B = 16
N_CORES = 8
B_LOC = B // N_CORES  # batches per core

P = 128          # SBUF/PSUM partitions
NCH = 8          # chunks over n / i (1024/128)
NH = 512         # psum free-dim half (one fp32 bank)

# "A": s/q/k bf16 (kq plain matmuls), att2 fp8 DoubleRow.
# "B": s/Qw/Kw fp8 -> kq matmuls DoubleRow too (half the PE columns + bytes).
MODE = "B"

_NC_CACHE = {}


def _build_nc(mode=MODE):
    import concourse.bass as bass  # noqa: F401
    import concourse.tile as tile
    from concourse import bacc, mybir

    f32 = mybir.dt.float32
    bf16 = mybir.dt.bfloat16
    f16 = mybir.dt.float16
    f8 = mybir.dt.float8e4
    AFT = mybir.ActivationFunctionType
    DR = mybir.MatmulPerfMode.DoubleRow
    ALU = mybir.AluOpType

    s_dt = f8 if mode == "B" else bf16
    w_dt = f8 if mode == "B" else bf16
    # mode B: Qw,Kw pre-scaled by 32 on host (fp8 denormal floor); att1
    # eviction multiplies by 2^-13 = 0.125/(32*32). mode A folds 0.125
    # into Qw directly, eviction scale 1.
    sq_scale = 2.0 ** -13 if mode == "B" else 1.0

    nc = bacc.Bacc(
        "TRN2",
        target_bir_lowering=False,
        debug=False,
        num_devices=N_CORES,
    )
    # All dram tensors are host-packed to match the SBUF tile layout, so
    # every DMA is a fully-contiguous per-partition transfer.
    sn_d = nc.dram_tensor("sn", [P, B_LOC, NCH, IN_DIM], s_dt, kind="ExternalInput")
    st_d = nc.dram_tensor("st", [P, B_LOC, NCH, N_DIM], s_dt, kind="ExternalInput")
    g_d = nc.dram_tensor("gmat", [P, NCH, IN_DIM], f8, kind="ExternalInput")
    qw_d = nc.dram_tensor("qw", [P, NCH, H_DIM], w_dt, kind="ExternalInput")
    kw_d = nc.dram_tensor("kw", [P, NCH, H_DIM], w_dt, kind="ExternalInput")
    o_d = nc.dram_tensor("out", [P, B_LOC, NCH, IN_DIM], f16, kind="ExternalOutput")

    with tile.TileContext(nc) as tc:
        with (
            tc.tile_pool(name="const", bufs=1) as const_pool,
            tc.tile_pool(name="gmat", bufs=1) as gmat_pool,
            tc.tile_pool(name="sn", bufs=2) as sn_pool,
            tc.tile_pool(name="st", bufs=2) as st_pool,
            tc.tile_pool(name="att1e", bufs=2) as att1e_pool,
            tc.tile_pool(name="att1q", bufs=2) as att1q_pool,
            tc.tile_pool(name="kq", bufs=2) as kq_pool,
            tc.tile_pool(name="outs", bufs=2) as out_pool,
            tc.tile_pool(name="stat", bufs=8) as stat_pool,
            tc.tile_pool(name="ps5", bufs=6, space="PSUM") as ps5,
            tc.tile_pool(name="psKQ", bufs=1, space="PSUM") as psKQ,
        ):
            qw_sb = const_pool.tile([P, NCH, H_DIM], w_dt)
            nc.sync.dma_start(qw_sb[:], qw_d.ap())
            kw_sb = const_pool.tile([P, NCH, H_DIM], w_dt)
            nc.sync.dma_start(kw_sb[:], kw_d.ap())

            g_sb = gmat_pool.tile([P, NCH, IN_DIM], f8)

            def phase_load_s(b):
                """Both layouts of s_b in 2 half-DMAs each (chunks 0-3, 4-7)."""
                sn_t = sn_pool.tile([P, NCH, IN_DIM], s_dt, tag="sn")
                st_t = st_pool.tile([P, NCH, N_DIM], s_dt, tag="st")
                halves = []
                for lo in (0, 4):
                    halves.append(nc.sync.dma_start(
                        sn_t[:, lo:lo + 4, :], sn_d.ap()[:, b, lo:lo + 4, :]
                    ))
                    halves.append(nc.sync.dma_start(
                        st_t[:, lo:lo + 4, :], st_d.ap()[:, b, lo:lo + 4, :]
                    ))
                return sn_t, st_t, halves

            def kq_matmuls(w_sb, s_t, ps):
                """PE instructions of one k or q chain (accum over chunks)."""
                ins = []
                if mode == "A":
                    for c in range(NCH):
                        for half in range(2):
                            ins.append(lambda c=c, half=half: nc.tensor.matmul(
                                ps[:, half * NH:(half + 1) * NH],
                                w_sb[:, c, :],
                                s_t[:, c, half * NH:(half + 1) * NH],
                                start=(c == 0),
                                stop=(c == NCH - 1),
                            ))
                else:
                    for ks in range(NCH // 2):
                        for half in range(2):
                            ins.append(lambda ks=ks, half=half: nc.tensor.matmul(
                                ps[:, half * NH:(half + 1) * NH],
                                w_sb[:, 2 * ks:2 * ks + 2, :],
                                s_t[:, 2 * ks:2 * ks + 2, half * NH:(half + 1) * NH],
                                start=(ks == 0),
                                stop=(ks == NCH // 2 - 1),
                                perf_mode=DR,
                            ))
                return ins

            def emit_evict_kq(ps, tag):
                sb = kq_pool.tile([H_DIM, N_DIM], bf16, tag=tag)
                nc.vector.tensor_copy(sb[:], ps[:])
                return sb

            att1_ctr = [0]

            def emit_att1_group(att1e, att1sq, k_sb, q_sb, ci, half):
                """att1T tile (ci,half): matmul then scaled Square -> fp8.
                Mostly on ACT (one fused instr from PSUM); every 8th tile on
                DVE (two instrs) to balance engine load."""
                pa = ps5.tile([P, NH], f32, tag="ps512")
                nc.tensor.matmul(
                    pa[:],
                    k_sb[:, ci * P:(ci + 1) * P],
                    q_sb[:, half * NH:(half + 1) * NH],
                    start=True,
                    stop=True,
                )
                dst = att1sq[:, ci, half * NH:(half + 1) * NH]
                att1_ctr[0] += 1
                if att1_ctr[0] % 8 != 0:
                    nc.scalar.activation(dst, pa[:], AFT.Square, scale=sq_scale)
                else:
                    stage = att1e[:, ci, half * NH:(half + 1) * NH]
                    nc.vector.tensor_scalar(
                        stage, pa[:], sq_scale, 0.0, op0=ALU.mult, op1=ALU.add,
                    )
                    nc.vector.tensor_mul(dst, stage, stage)

            def phase_att2_group(b, att1sq, out_sb, nt):
                """One att2 row-block: DoubleRow matmuls, ACT evict+rowsums,
                DVE 1/(rs+1e-3), GPSIMD normalize in place."""
                po0 = ps5.tile([P, NH], f32, tag="ps512")
                po1 = ps5.tile([P, NH], f32, tag="ps512")
                for ks in range(NCH // 2):
                    lhsT = att1sq[:, 2 * ks:2 * ks + 2, nt * P:(nt + 1) * P]
                    nc.tensor.matmul(
                        po0[:], lhsT, g_sb[:, 2 * ks:2 * ks + 2, 0:NH],
                        start=(ks == 0), stop=(ks == NCH // 2 - 1),
                        perf_mode=DR,
                    )
                    nc.tensor.matmul(
                        po1[:], lhsT, g_sb[:, 2 * ks:2 * ks + 2, NH:2 * NH],
                        start=(ks == 0), stop=(ks == NCH // 2 - 1),
                        perf_mode=DR,
                    )
                dst = out_sb[:, nt, :]
                rs0 = stat_pool.tile([P, 1], f32, tag="rs0")
                rs1 = stat_pool.tile([P, 1], f32, tag="rs1")
                # evict the two halves on different engines in parallel
                nc.scalar.activation(dst[:, 0:NH], po0[:], AFT.Copy, accum_out=rs0[:])
                nc.vector.tensor_scalar(
                    dst[:, NH:2 * NH], po1[:], 1.0, 0.0,
                    op0=ALU.mult, op1=ALU.add, accum_out=rs1[:],
                )
                rinv = stat_pool.tile([P, 1], f32, tag="rinv")
                nc.vector.scalar_tensor_tensor(
                    rinv[:], rs0[:], 1e-3, rs1[:], op0=ALU.add, op1=ALU.add,
                )
                nc.vector.reciprocal(rinv[:], rinv[:])
                nc.vector.tensor_scalar_mul(dst, dst, rinv[:])

            def emit_out_dma(b, out_sb, lo):
                # second HWDGE ring (ACT-issued) so outputs don't block inputs
                nc.scalar.dma_start(
                    o_d.ap()[:, b, lo:lo + 4, :], out_sb[:, lo:lo + 4, :]
                )

            ATT1_ORDER = [(ci, half) for half in range(2) for ci in range(NCH)]

            # ---- DMA schedule (one FIFO ring): s(b0) -> G -> s(b1)
            sn0, st0, _ = phase_load_s(0)
            nc.sync.dma_start(g_sb[:], g_d.ap())
            sn1, st1, _ = phase_load_s(1)

            # ---- batch 0: k, q, att1
            ps_k0 = psKQ.tile([H_DIM, N_DIM], f32, tag="kq")
            for m in kq_matmuls(kw_sb, sn0, ps_k0):
                m()
            k_sb0 = emit_evict_kq(ps_k0, "k")
            ps_q0 = psKQ.tile([H_DIM, N_DIM], f32, tag="kq")
            for m in kq_matmuls(qw_sb, st0, ps_q0):
                m()
            q_sb0 = emit_evict_kq(ps_q0, "q")
            att1e0 = att1e_pool.tile([P, NCH, N_DIM], bf16, tag="att1e")
            att1sq0 = att1q_pool.tile([P, NCH, N_DIM], f8, tag="att1q")
            for ci, half in ATT1_ORDER:
                emit_att1_group(att1e0, att1sq0, k_sb0, q_sb0, ci, half)

            # ---- att2(b0) with k/q/att1 of batch 1 woven into the stream
            out_sb0 = out_pool.tile([P, NCH, IN_DIM], f16, tag="out")
            ps_k1 = psKQ.tile([H_DIM, N_DIM], f32, tag="kq")
            k_ins = kq_matmuls(kw_sb, sn1, ps_k1)
            nk = len(k_ins)
            att1e1 = att1e_pool.tile([P, NCH, N_DIM], bf16, tag="att1e")
            att1sq1 = att1q_pool.tile([P, NCH, N_DIM], f8, tag="att1q")
            holder = {}

            def weave_q_start():
                ps_q1 = psKQ.tile([H_DIM, N_DIM], f32, tag="kq")
                holder["q_ins"] = kq_matmuls(qw_sb, st1, ps_q1)
                holder["ps_q1"] = ps_q1

            for nt in range(NCH):
                phase_att2_group(0, att1sq0, out_sb0, nt)
                if nt == 3:
                    emit_out_dma(0, out_sb0, 0)
                if nt == 1:
                    for m in k_ins[0:nk // 2]:
                        m()
                elif nt == 2:
                    for m in k_ins[nk // 2:]:
                        m()
                    holder["k_sb1"] = emit_evict_kq(ps_k1, "k")
                    weave_q_start()
                elif nt == 3:
                    for m in holder["q_ins"][0:nk // 2]:
                        m()
                elif nt == 4:
                    for m in holder["q_ins"][nk // 2:]:
                        m()
                    holder["q_sb1"] = emit_evict_kq(holder["ps_q1"], "q")
                elif nt == 5:
                    for ci, half in ATT1_ORDER[0:5]:
                        emit_att1_group(att1e1, att1sq1,
                                        holder["k_sb1"], holder["q_sb1"], ci, half)
                elif nt == 6:
                    for ci, half in ATT1_ORDER[5:10]:
                        emit_att1_group(att1e1, att1sq1,
                                        holder["k_sb1"], holder["q_sb1"], ci, half)
                elif nt == 7:
                    for ci, half in ATT1_ORDER[10:16]:
                        emit_att1_group(att1e1, att1sq1,
                                        holder["k_sb1"], holder["q_sb1"], ci, half)
            emit_out_dma(0, out_sb0, 4)

            out_sb1 = out_pool.tile([P, NCH, IN_DIM], f16, tag="out")
            for nt in range(NCH):
                phase_att2_group(1, att1sq1, out_sb1, nt)
                if nt == 3:
                    emit_out_dma(1, out_sb1, 0)
            emit_out_dma(1, out_sb1, 4)

    nc.compile()
    return nc


def _get_nc(mode=MODE):
    if mode not in _NC_CACHE:
        _NC_CACHE[mode] = _build_nc(mode)
    return _NC_CACHE[mode]


def _pack(a):
    """[C*P, ...tail] -> [P, ...tail-with-C-inserted]: rows c*P+p -> [p][c]."""
    c = a.shape[0] // P
    return np.ascontiguousarray(
        a.reshape(c, P, *a.shape[1:]).swapaxes(0, 1)
    )


def _run(inputs, trace=False, mm_mode=None, tmpdir=None, mode=MODE):
    import ml_dtypes
    from concourse.bass_utils import run_bass_kernel_spmd

    bf16 = ml_dtypes.bfloat16
    f8 = ml_dtypes.float8_e4m3
    s_np = f8 if mode == "B" else bf16
    w_np = f8 if mode == "B" else bf16
    q_mult = 32.0 if mode == "B" else 0.125
    k_mult = 32.0 if mode == "B" else 1.0

    s = np.asarray(inputs["s"], dtype=np.float32)
    # pack to [P, NCH, IN_DIM]: row (c*128+p) -> [p, c]
    g8 = _pack(np.asarray(inputs["Gmat"], np.float32).astype(f8))
    qw = _pack((np.asarray(inputs["Qweight"], np.float32) * q_mult).astype(w_np))
    kw = _pack((np.asarray(inputs["Kweight"], np.float32) * k_mult).astype(w_np))

    s_c = s.astype(s_np)  # cast once, slice per core

    nc = _get_nc(mode)
    in_maps = []
    for c in range(N_CORES):
        sl = s_c[:, c * B_LOC:(c + 1) * B_LOC, :]        # [N, B_LOC, I]
        # sn[p, b, ch, i] = sl[ch*128+p, b, i]
        sn = _pack(sl).swapaxes(1, 2)                     # [P, B_LOC, NCH, I]
        # st[p, b, ch, n] = sl[n, b, ch*128+p]
        st = _pack(np.ascontiguousarray(sl.transpose(2, 1, 0))).swapaxes(1, 2)
        in_maps.append({
            "sn": np.ascontiguousarray(sn),
            "st": np.ascontiguousarray(st),
            "gmat": g8,
            "qw": qw,
            "kw": kw,
        })
    res = run_bass_kernel_spmd(
        nc, in_maps, list(range(N_CORES)), trace=trace, tmpdir=tmpdir
    )
    outs = []
    for c in range(N_CORES):
        o = res.results[c]["out"]                         # [P, B_LOC, NCH, I] f16
        # out[ch*128+p, b, j] = o[p, b, ch, j]
        outs.append(o.transpose(2, 0, 1, 3).reshape(N_DIM, B_LOC, IN_DIM))
    out = np.concatenate(outs, axis=1).astype(np.float32)
    return out, res


def kernel(**inputs) -> np.ndarray:
    out, _ = _run(inputs, trace=False)
    return out


# revision 13
# speedup vs baseline: 3.6297x; 1.0333x over previous
"""Trainium2 Bass kernel for nn_GAttention (gnn_message_passing).

Reference computation (per batch b):
    q = s[:,b,:] @ Qweight                      # (N, H)
    k = Kweight.T @ s[:,b,:]                    # (H, I)   (contraction over n)
    att1 = (q @ k) * (1/sqrt(H)) + 1e-9         # (N, I)
    att2 = att1**2 @ Gmat                       # (N, I)
    out[:,b,:] = att2 / (rowsum(att2) + 1e-3)

Sharding: pure data-parallel over batch B=16 -> 2 batches per core on 8 cores.
Gmat/Qweight/Kweight replicated.

Measured bottlenecks this kernel is built around (from ntff profiles):
  * DMA is issue-rate bound: ~8ns/packet on one HWDGE ring regardless of
    packet size, so every tensor is HOST-PACKED into its exact SBUF tile
    layout and moved with few, large, fully-contiguous dma_starts
    (>=512KB each, ~340GB/s instead of ~170GB/s chunked).
  * The PE streams ~1 moving column/cycle regardless of dtype; fp8
    MatmulPerfMode.DoubleRow doubles K per column (measured 2x vs bf16).
    So s ships in fp8 e4m3 in BOTH layouts (s_n for k, s_T for q) and
    k/q/att2 all run DoubleRow fp8. att1 (K=64 only) stays bf16.
  * Qweight/Kweight are pre-scaled by 32 on the host so their values sit
    in fp8's normal range; the compensating 2^-13 (= (1/8)/32/32) is
    folded into the att1 PSUM eviction (exact power-of-two scales).
    The +1e-9 inside the square is dropped (relative ~3e-9).
  * Vector-engine work is spread over three engines: ACT evicts att2
    PSUM->f16 with fused accum_out rowsums, DVE evicts att1 PSUM->bf16
    (scaled) and builds 1/(rs0+rs1+1e-3), GPSIMD (SBUF-only engine)
    squares att1 (bf16->fp8) and applies the row normalization in place.
  * Outputs stage per-batch in SBUF f16 and leave in 1MB DMAs on the
    second HWDGE ring (nc.scalar) so they never block input streaming.

The two batches are software-pipelined: batch 1's k/q/att1 phases are
woven into batch 0's att2 group stream so the PE never idles long enough
for the HAM clock gate to throttle.
"""

import sys

import numpy as np

try:  # concourse normally comes from the image's NIX_PYTHONPATH
    import concourse  # noqa: F401
except ImportError:  # pragma: no cover
    sys.path.insert(0, "/opt/trn_rl_repo")

N_DIM = 1024
IN_DIM = 1024
H_DIM = 64
B = 16
N_CORES = 8
B_LOC = B // N_CORES  # batches per core

P = 128          # SBUF/PSUM partitions
NCH = 8          # chunks over n / i (1024/128)
NH = 512         # psum free-dim half (one fp32 bank)

# "A": s/q/k bf16 (kq plain matmuls), att2 fp8 DoubleRow.
# "B": s/Qw/Kw fp8 -> kq matmuls DoubleRow too (half the PE columns + bytes).
MODE = "B"

_NC_CACHE = {}


def _build_nc(mode=MODE):
    import concourse.bass as bass  # noqa: F401
    import concourse.tile as tile
    from concourse import bacc, mybir

    f32 = mybir.dt.float32
    bf16 = mybir.dt.bfloat16
    f16 = mybir.dt.float16
    f8 = mybir.dt.float8e4
    AFT = mybir.ActivationFunctionType
    DR = mybir.MatmulPerfMode.DoubleRow
    ALU = mybir.AluOpType

    s_dt = f8 if mode == "B" else bf16
    w_dt = f8 if mode == "B" else bf16
    # mode B: Qw,Kw pre-scaled by 32 on host (fp8 denormal floor); att1
    # eviction multiplies by 2^-13 = 0.125/(32*32). mode A folds 0.125
    # into Qw directly, eviction scale 1.
    sq_scale = 2.0 ** -13 if mode == "B" else 1.0

    nc = bacc.Bacc(
        "TRN2",
        target_bir_lowering=False,
        debug=False,
        num_devices=N_CORES,
    )
    # All dram tensors are host-packed to match the SBUF tile layout, so
    # every DMA is a fully-contiguous per-partition transfer.
    sn_d = nc.dram_tensor("sn", [P, B_LOC, NCH, IN_DIM], s_dt, kind="ExternalInput")
    st_d = nc.dram_tensor("st", [P, B_LOC, NCH, N_DIM], s_dt, kind="ExternalInput")
    g_d = nc.dram_tensor("gmat", [P, NCH, IN_DIM], f8, kind="ExternalInput")
    qw_d = nc.dram_tensor("qw", [P, NCH, H_DIM], w_dt, kind="ExternalInput")
    kw_d = nc.dram_tensor("kw", [P, NCH, H_DIM], w_dt, kind="ExternalInput")
    o_d = nc.dram_tensor("out", [P, B_LOC, NCH, IN_DIM], f16, kind="ExternalOutput")

    with tile.TileContext(nc) as tc:
        with (
            tc.tile_pool(name="const", bufs=1) as const_pool,
            tc.tile_pool(name="gmat", bufs=1) as gmat_pool,
            tc.tile_pool(name="sn", bufs=2) as sn_pool,
            tc.tile_pool(name="st", bufs=2) as st_pool,
            tc.tile_pool(name="att1e", bufs=2) as att1e_pool,
            tc.tile_pool(name="att1q", bufs=2) as att1q_pool,
            tc.tile_pool(name="kq", bufs=2) as kq_pool,
            tc.tile_pool(name="outs", bufs=2) as out_pool,
            tc.tile_pool(name="stat", bufs=8) as stat_pool,
            tc.tile_pool(name="ps5", bufs=6, space="PSUM") as ps5,
            tc.tile_pool(name="psKQ", bufs=1, space="PSUM") as psKQ,
        ):
            qw_sb = const_pool.tile([P, NCH, H_DIM], w_dt)
            kw_sb = const_pool.tile([P, NCH, H_DIM], w_dt)
            g_sb = gmat_pool.tile([P, NCH, IN_DIM], f8)

            def phase_load_s(b, first=False):
                """Both layouts of s_b in 2 half-DMAs each (chunks 0-3, 4-7).
                For batch 0 the weight DMAs are woven in so the k-chain's
                inputs (sn chunks + kw) all arrive as early as possible."""
                sn_t = sn_pool.tile([P, NCH, IN_DIM], s_dt, tag="sn")
                st_t = st_pool.tile([P, NCH, N_DIM], s_dt, tag="st")
                nc.sync.dma_start(sn_t[:, 0:4, :], sn_d.ap()[:, b, 0:4, :])
                if first:
                    nc.sync.dma_start(kw_sb[:], kw_d.ap())
                nc.sync.dma_start(sn_t[:, 4:8, :], sn_d.ap()[:, b, 4:8, :])
                if first:
                    nc.sync.dma_start(qw_sb[:], qw_d.ap())
                nc.sync.dma_start(st_t[:, 0:4, :], st_d.ap()[:, b, 0:4, :])
                nc.sync.dma_start(st_t[:, 4:8, :], st_d.ap()[:, b, 4:8, :])
                return sn_t, st_t

            def kq_matmuls(w_sb, s_t, ps):
                """PE instructions of one k or q chain (accum over chunks)."""
                ins = []
                if mode == "A":
                    for c in range(NCH):
                        for half in range(2):
                            ins.append(lambda c=c, half=half: nc.tensor.matmul(
                                ps[:, half * NH:(half + 1) * NH],
                                w_sb[:, c, :],
                                s_t[:, c, half * NH:(half + 1) * NH],
                                start=(c == 0),
                                stop=(c == NCH - 1),
                            ))
                else:
                    for ks in range(NCH // 2):
                        for half in range(2):
                            ins.append(lambda ks=ks, half=half: nc.tensor.matmul(
                                ps[:, half * NH:(half + 1) * NH],
                                w_sb[:, 2 * ks:2 * ks + 2, :],
                                s_t[:, 2 * ks:2 * ks + 2, half * NH:(half + 1) * NH],
                                start=(ks == 0),
                                stop=(ks == NCH // 2 - 1),
                                perf_mode=DR,
                            ))
                return ins

            def emit_evict_kq(ps, tag):
                sb = kq_pool.tile([H_DIM, N_DIM], bf16, tag=tag)
                nc.vector.tensor_copy(sb[:], ps[:])
                return sb

            att1_ctr = [0]

            def emit_att1_group(att1e, att1sq, k_sb, q_sb, ci, half, dve_every):
                """att1T tile (ci,half): matmul then scaled Square -> fp8.
                Mostly on ACT (one fused instr from PSUM); every dve_every-th
                tile goes to DVE (two instrs) to balance engine load."""
                pa = ps5.tile([P, NH], f32, tag="ps512")
                nc.tensor.matmul(
                    pa[:],
                    k_sb[:, ci * P:(ci + 1) * P],
                    q_sb[:, half * NH:(half + 1) * NH],
                    start=True,
                    stop=True,
                )
                dst = att1sq[:, ci, half * NH:(half + 1) * NH]
                att1_ctr[0] += 1
                if att1_ctr[0] % dve_every != 0:
                    nc.scalar.activation(dst, pa[:], AFT.Square, scale=sq_scale)
                else:
                    stage = att1e[:, ci, half * NH:(half + 1) * NH]
                    nc.vector.tensor_scalar(
                        stage, pa[:], sq_scale, 0.0, op0=ALU.mult, op1=ALU.add,
                    )
                    nc.vector.tensor_mul(dst, stage, stage)

            def phase_att2_group(b, att1sq, out_sb, nt):
                """One att2 row-block: DoubleRow matmuls, ACT evict+rowsums,
                DVE 1/(rs+1e-3), GPSIMD normalize in place."""
                po0 = ps5.tile([P, NH], f32, tag="ps512")
                po1 = ps5.tile([P, NH], f32, tag="ps512")
                for ks in range(NCH // 2):
                    lhsT = att1sq[:, 2 * ks:2 * ks + 2, nt * P:(nt + 1) * P]
                    nc.tensor.matmul(
                        po0[:], lhsT, g_sb[:, 2 * ks:2 * ks + 2, 0:NH],
                        start=(ks == 0), stop=(ks == NCH // 2 - 1),
                        perf_mode=DR,
                    )
                    nc.tensor.matmul(
                        po1[:], lhsT, g_sb[:, 2 * ks:2 * ks + 2, NH:2 * NH],
                        start=(ks == 0), stop=(ks == NCH // 2 - 1),
                        perf_mode=DR,
                    )
                dst = out_sb[:, nt, :]
                rs0 = stat_pool.tile([P, 1], f32, tag="rs0")
                rs1 = stat_pool.tile([P, 1], f32, tag="rs1")
                # evict the two halves on different engines in parallel
                nc.scalar.activation(dst[:, 0:NH], po0[:], AFT.Copy, accum_out=rs0[:])
                nc.vector.tensor_scalar(
                    dst[:, NH:2 * NH], po1[:], 1.0, 0.0,
                    op0=ALU.mult, op1=ALU.add, accum_out=rs1[:],
                )
                rinv = stat_pool.tile([P, 1], f32, tag="rinv")
                nc.vector.scalar_tensor_tensor(
                    rinv[:], rs0[:], 1e-3, rs1[:], op0=ALU.add, op1=ALU.add,
                )
                nc.vector.reciprocal(rinv[:], rinv[:])
                nc.vector.tensor_scalar_mul(dst, dst, rinv[:])

            def emit_out_dma(b, out_sb, lo, hi):
                # GPSIMD queue is otherwise idle, so the DMA trigger's
                # semaphore wait never blocks compute instructions
                nc.gpsimd.dma_start(
                    o_d.ap()[:, b, lo:hi, :], out_sb[:, lo:hi, :]
                )

            ATT1_ORDER = [(ci, half) for half in range(2) for ci in range(NCH)]

            # ---- DMA schedule (one FIFO ring): s(b0)+weights -> G -> s(b1)
            sn0, st0 = phase_load_s(0, first=True)
            nc.sync.dma_start(g_sb[:], g_d.ap())
            sn1, st1 = phase_load_s(1)

            # ---- batch 0: k, q, att1
            ps_k0 = psKQ.tile([H_DIM, N_DIM], f32, tag="kq")
            for m in kq_matmuls(kw_sb, sn0, ps_k0):
                m()
            k_sb0 = emit_evict_kq(ps_k0, "k")
            ps_q0 = psKQ.tile([H_DIM, N_DIM], f32, tag="kq")
            for m in kq_matmuls(qw_sb, st0, ps_q0):
                m()
            q_sb0 = emit_evict_kq(ps_q0, "q")
            att1e0 = att1e_pool.tile([P, NCH, N_DIM], bf16, tag="att1e")
            att1sq0 = att1q_pool.tile([P, NCH, N_DIM], f8, tag="att1q")
            for ci, half in ATT1_ORDER:
                emit_att1_group(att1e0, att1sq0, k_sb0, q_sb0, ci, half, 3)

            # ---- att2(b0) with k/q/att1 of batch 1 woven into the stream
            out_sb0 = out_pool.tile([P, NCH, IN_DIM], f16, tag="out")
            ps_k1 = psKQ.tile([H_DIM, N_DIM], f32, tag="kq")
            k_ins = kq_matmuls(kw_sb, sn1, ps_k1)
            nk = len(k_ins)
            att1e1 = att1e_pool.tile([P, NCH, N_DIM], bf16, tag="att1e")
            att1sq1 = att1q_pool.tile([P, NCH, N_DIM], f8, tag="att1q")
            holder = {}

            def weave_q_start():
                ps_q1 = psKQ.tile([H_DIM, N_DIM], f32, tag="kq")
                holder["q_ins"] = kq_matmuls(qw_sb, st1, ps_q1)
                holder["ps_q1"] = ps_q1

            for nt in range(NCH):
                phase_att2_group(0, att1sq0, out_sb0, nt)
                if nt == 3:
                    emit_out_dma(0, out_sb0, 0, 4)
                elif nt == 5:
                    emit_out_dma(0, out_sb0, 4, 6)
                if nt == 1:
                    for m in k_ins[0:nk // 2]:
                        m()
                elif nt == 2:
                    for m in k_ins[nk // 2:]:
                        m()
                    holder["k_sb1"] = emit_evict_kq(ps_k1, "k")
                    weave_q_start()
                elif nt == 3:
                    for m in holder["q_ins"][0:nk // 2]:
                        m()
                elif nt == 4:
                    for m in holder["q_ins"][nk // 2:]:
                        m()
                    holder["q_sb1"] = emit_evict_kq(holder["ps_q1"], "q")
                elif nt == 5:
                    for ci, half in ATT1_ORDER[0:5]:
                        emit_att1_group(att1e1, att1sq1,
                                        holder["k_sb1"], holder["q_sb1"], ci, half, 8)
                elif nt == 6:
                    for ci, half in ATT1_ORDER[5:10]:
                        emit_att1_group(att1e1, att1sq1,
                                        holder["k_sb1"], holder["q_sb1"], ci, half, 8)
                elif nt == 7:
                    for ci, half in ATT1_ORDER[10:16]:
                        emit_att1_group(att1e1, att1sq1,
                                        holder["k_sb1"], holder["q_sb1"], ci, half, 8)
            emit_out_dma(0, out_sb0, 6, 8)

            out_sb1 = out_pool.tile([P, NCH, IN_DIM], f16, tag="out")
            for nt in range(NCH):
                phase_att2_group(1, att1sq1, out_sb1, nt)
                if nt == 3:
                    emit_out_dma(1, out_sb1, 0, 4)
                elif nt == 5:
                    emit_out_dma(1, out_sb1, 4, 6)
            emit_out_dma(1, out_sb1, 6, 8)

    nc.compile()
    return nc


def _get_nc(mode=MODE):
    if mode not in _NC_CACHE:
        _NC_CACHE[mode] = _build_nc(mode)
    return _NC_CACHE[mode]


def _pack(a):
    """[C*P, ...tail] -> [P, ...tail-with-C-inserted]: rows c*P+p -> [p][c]."""
    c = a.shape[0] // P
    return np.ascontiguousarray(
        a.reshape(c, P, *a.shape[1:]).swapaxes(0, 1)
    )


def _run(inputs, trace=False, mm_mode=None, tmpdir=None, mode=MODE):
    import ml_dtypes
    from concourse.bass_utils import run_bass_kernel_spmd

    bf16 = ml_dtypes.bfloat16
    f8 = ml_dtypes.float8_e4m3
    s_np = f8 if mode == "B" else bf16
    w_np = f8 if mode == "B" else bf16
    q_mult = 32.0 if mode == "B" else 0.125
    k_mult = 32.0 if mode == "B" else 1.0

    s = np.asarray(inputs["s"], dtype=np.float32)
    # pack to [P, NCH, IN_DIM]: row (c*128+p) -> [p, c]
    g8 = _pack(np.asarray(inputs["Gmat"], np.float32).astype(f8))
    qw = _pack((np.asarray(inputs["Qweight"], np.float32) * q_mult).astype(w_np))
    kw = _pack((np.asarray(inputs["Kweight"], np.float32) * k_mult).astype(w_np))

    s_c = s.astype(s_np)  # cast once, slice per core

    nc = _get_nc(mode)
    in_maps = []
    for c in range(N_CORES):
        sl = s_c[:, c * B_LOC:(c + 1) * B_LOC, :]        # [N, B_LOC, I]
        # sn[p, b, ch, i] = sl[ch*128+p, b, i]
        sn = _pack(sl).swapaxes(1, 2)                     # [P, B_LOC, NCH, I]
        # st[p, b, ch, n] = sl[n, b, ch*128+p]
        st = _pack(np.ascontiguousarray(sl.transpose(2, 1, 0))).swapaxes(1, 2)
        in_maps.append({
            "sn": np.ascontiguousarray(sn),
            "st": np.ascontiguousarray(st),
            "gmat": g8,
            "qw": qw,
            "kw": kw,
        })
    res = run_bass_kernel_spmd(
        nc, in_maps, list(range(N_CORES)), trace=trace, tmpdir=tmpdir
    )
    outs = []
    for c in range(N_CORES):
        o = res.results[c]["out"]                         # [P, B_LOC, NCH, I] f16
        # out[ch*128+p, b, j] = o[p, b, ch, j]
        outs.append(o.transpose(2, 0, 1, 3).reshape(N_DIM, B_LOC, IN_DIM))
    out = np.concatenate(outs, axis=1).astype(np.float32)
    return out, res


def kernel(**inputs) -> np.ndarray:
    out, _ = _run(inputs, trace=False)
    return out


# revision 15
# speedup vs baseline: 4.0075x; 1.1041x over previous
"""Trainium2 Bass kernel for nn_GAttention (gnn_message_passing).

Reference computation (per batch b):
    q = s[:,b,:] @ Qweight                      # (N, H)
    k = Kweight.T @ s[:,b,:]                    # (H, I)   (contraction over n)
    att1 = (q @ k) * (1/sqrt(H)) + 1e-9         # (N, I)
    att2 = att1**2 @ Gmat                       # (N, I)
    out[:,b,:] = att2 / (rowsum(att2) + 1e-3)

Sharding: pure data-parallel over batch B=16 -> 2 batches per core on 8 cores.
Gmat/Qweight/Kweight replicated.

Design, driven by ntff profiles of earlier revisions:
  * DMA: every tensor is HOST-PACKED into its exact SBUF tile layout and
    moved in few large fully-contiguous dma_starts (~340GB/s vs ~170
    chunked). Inputs ride the sync HWDGE ring, outputs ride it too but
    only after all inputs are issued; their semaphore waits sit in
    otherwise-idle queues so they never block compute.
  * PE streams one moving column per cycle (~216ns per 512-wide matmul
    at 2.4GHz); fp8 DoubleRow doubles the contraction per column.
    M=64 matmuls run at half rate, so k and q are FUSED into a single
    M=128 DoubleRow chain with block-diagonal weights:
        subtile0 = [Kw | 0] paired with s_n,  subtile1 = [0 | Qw] with s_T
    -> PSUM partitions 0-63 = k, 64-127 = q, full-rate.
    s ships fp8 e4m3 with s_n/s_T chunks interleaved in one tensor.
  * att1 (K=64) would idle half the PE rows; after two SBUF->SBUF DMA
    partition remaps (building the swapped copy of k/q), att1 tiles run
    as CONCURRENT row-tiled pairs: rows 0-63 and rows 64-127.
  * Qweight/Kweight are pre-scaled by 32 on the host (fp8 normal range);
    the compensating 2^-13 = (1/8)/32/32 is applied in the att1
    eviction / square (exact powers of two). The +1e-9 is dropped
    (relative ~3e-9).
  * Vector work is spread: ACT evicts att2 po0 PSUM->f16 with fused
    accum_out rowsum + most att1 squares; DVE evicts po1 the same way +
    reciprocal + row normalization; GPSIMD adds rs0+1e-3+rs1 and issues
    nothing else heavy (its tensor ops are software-slow).

The two batches are software-pipelined: batch 1's kq/att1 phases are
woven into batch 0's att2 group stream so the PE never idles long
enough for the HAM clock gate to throttle.
"""

import sys

import numpy as np

try:  # concourse normally comes from the image's NIX_PYTHONPATH
    import concourse  # noqa: F401
except ImportError:  # pragma: no cover
    sys.path.insert(0, "/opt/trn_rl_repo")

N_DIM = 1024
IN_DIM = 1024
H_DIM = 64
B = 16
N_CORES = 8
B_LOC = B // N_CORES  # batches per core

P = 128          # SBUF/PSUM partitions
NCH = 8          # chunks over n / i (1024/128)
NH = 512         # psum free-dim half (one fp32 bank)

MODE = "B"       # fp8 everywhere the PE touches; "A" (bf16 feeds) unused

_NC_CACHE = {}


def _build_nc(mode=MODE):
    import concourse.bass as bass  # noqa: F401
    import concourse.tile as tile
    from concourse import bacc, mybir

    f32 = mybir.dt.float32
    bf16 = mybir.dt.bfloat16
    f16 = mybir.dt.float16
    f8 = mybir.dt.float8e4
    AFT = mybir.ActivationFunctionType
    DR = mybir.MatmulPerfMode.DoubleRow
    ALU = mybir.AluOpType

    assert mode == "B"
    sq_scale = 2.0 ** -13  # (1/8) / (32*32)

    nc = bacc.Bacc(
        "TRN2",
        target_bir_lowering=False,
        debug=False,
        num_devices=N_CORES,
    )
    # host-packed layouts (see _run): s has s_n / s_T chunk-interleaved
    s_d = nc.dram_tensor("s", [P, B_LOC, NCH, 2, N_DIM], f8, kind="ExternalInput")
    g_d = nc.dram_tensor("gmat", [P, NCH, IN_DIM], f8, kind="ExternalInput")
    w_d = nc.dram_tensor("wkq", [P, NCH, 2, P], f8, kind="ExternalInput")
    o_d = nc.dram_tensor("out", [P, B_LOC, NCH, IN_DIM], f16, kind="ExternalOutput")

    with tile.TileContext(nc) as tc:
        with (
            tc.tile_pool(name="const", bufs=1) as const_pool,
            tc.tile_pool(name="gmat", bufs=1) as gmat_pool,
            tc.tile_pool(name="sb", bufs=2) as s_pool,
            tc.tile_pool(name="att1e", bufs=2) as att1e_pool,
            tc.tile_pool(name="att1q", bufs=2) as att1q_pool,
            tc.tile_pool(name="kq", bufs=2) as kq_pool,
            tc.tile_pool(name="kqs", bufs=2) as kqs_pool,
            tc.tile_pool(name="outs", bufs=2) as out_pool,
            tc.tile_pool(name="stat", bufs=8) as stat_pool,
            tc.tile_pool(name="ps5", bufs=6, space="PSUM") as ps5,
            tc.tile_pool(name="psKQ", bufs=1, space="PSUM") as psKQ,
        ):
            w_sb = const_pool.tile([P, NCH, 2, P], f8)
            g_sb = gmat_pool.tile([P, NCH, IN_DIM], f8)

            def phase_load_s(b, first=False):
                """One interleaved tensor per batch, 2 half-DMAs."""
                s_t = s_pool.tile([P, NCH, 2, N_DIM], f8, tag="s")
                nc.sync.dma_start(s_t[:, 0:4, :, :], s_d.ap()[:, b, 0:4, :, :])
                if first:
                    nc.sync.dma_start(w_sb[:], w_d.ap())
                nc.sync.dma_start(s_t[:, 4:8, :, :], s_d.ap()[:, b, 4:8, :, :])
                return s_t

            def kq_matmuls(s_t, ps):
                """Fused k+q chain: block-diagonal DoubleRow, M=128.
                psum rows 0-63 = k[h, :], rows 64-127 = qT[h, :]."""
                ins = []
                for c in range(NCH):
                    for half in range(2):
                        ins.append(lambda c=c, half=half: nc.tensor.matmul(
                            ps[:, half * NH:(half + 1) * NH],
                            w_sb[:, c, :, :],
                            s_t[:, c, :, half * NH:(half + 1) * NH],
                            start=(c == 0),
                            stop=(c == NCH - 1),
                            perf_mode=DR,
                        ))
                return ins

            def emit_evict_kq(ps):
                """Evict fused kq psum, then build the row-swapped copy via
                two SBUF->SBUF partition-remap DMAs (sync ring, idle then).
                kq_t rows: 0-63 k, 64-127 q;  kq_s rows: 0-63 q, 64-127 k."""
                kq_t = kq_pool.tile([P, N_DIM], bf16, tag="kq")
                nc.vector.tensor_copy(kq_t[:], ps[:])
                kq_s = kqs_pool.tile([P, N_DIM], bf16, tag="kqs")
                nc.sync.dma_start(kq_s[0:H_DIM, :], kq_t[H_DIM:P, :])
                nc.sync.dma_start(kq_s[H_DIM:P, :], kq_t[0:H_DIM, :])
                return kq_t, kq_s

            att1_ctr = [0]

            def emit_att1_pair(att1e, att1sq, kq_t, kq_s, ci, half, dve_every):
                """Two att1 tiles (ci,half),(ci+1,half) as a concurrent
                row-tiled pair: rows 0-63 (k_lo x q_lo) and 64-127
                (k_hi x q_hi). Squares mostly on ACT, some on DVE."""
                pa0 = ps5.tile([P, NH], f32, tag="ps512")
                pa1 = ps5.tile([P, NH], f32, tag="ps512")
                nc.tensor.matmul(
                    pa0[:],
                    kq_t[0:H_DIM, ci * P:(ci + 1) * P],
                    kq_s[0:H_DIM, half * NH:(half + 1) * NH],
                    start=True, stop=True, tile_position=(0, 0),
                )
                nc.tensor.matmul(
                    pa1[:],
                    kq_s[H_DIM:P, (ci + 1) * P:(ci + 2) * P],
                    kq_t[H_DIM:P, half * NH:(half + 1) * NH],
                    start=True, stop=True, tile_position=(H_DIM, 0),
                )
                for cc, pa in ((ci, pa0), (ci + 1, pa1)):
                    dst = att1sq[:, cc, half * NH:(half + 1) * NH]
                    att1_ctr[0] += 1
                    if att1_ctr[0] % dve_every != 0:
                        nc.scalar.activation(dst, pa[:], AFT.Square, scale=sq_scale)
                    else:
                        stage = att1e[:, cc, half * NH:(half + 1) * NH]
                        nc.vector.tensor_scalar(
                            stage, pa[:], sq_scale, 0.0, op0=ALU.mult, op1=ALU.add,
                        )
                        nc.vector.tensor_mul(dst, stage, stage)

            def phase_att2_group(b, att1sq, out_sb, nt):
                """One att2 row-block: DoubleRow matmuls; ACT/DVE evict the
                two PSUM halves in parallel with fused rowsums; GPSIMD adds
                rs0+1e-3+rs1; DVE reciprocal + row normalization."""
                po0 = ps5.tile([P, NH], f32, tag="ps512")
                po1 = ps5.tile([P, NH], f32, tag="ps512")
                for ks in range(NCH // 2):
                    lhsT = att1sq[:, 2 * ks:2 * ks + 2, nt * P:(nt + 1) * P]
                    nc.tensor.matmul(
                        po0[:], lhsT, g_sb[:, 2 * ks:2 * ks + 2, 0:NH],
                        start=(ks == 0), stop=(ks == NCH // 2 - 1),
                        perf_mode=DR,
                    )
                    nc.tensor.matmul(
                        po1[:], lhsT, g_sb[:, 2 * ks:2 * ks + 2, NH:2 * NH],
                        start=(ks == 0), stop=(ks == NCH // 2 - 1),
                        perf_mode=DR,
                    )
                dst = out_sb[:, nt, :]
                rs0 = stat_pool.tile([P, 1], f32, tag="rs0")
                rs1 = stat_pool.tile([P, 1], f32, tag="rs1")
                nc.scalar.activation(dst[:, 0:NH], po0[:], AFT.Copy, accum_out=rs0[:])
                nc.vector.tensor_scalar(
                    dst[:, NH:2 * NH], po1[:], 1.0, 0.0,
                    op0=ALU.mult, op1=ALU.add, accum_out=rs1[:],
                )
                rinv = stat_pool.tile([P, 1], f32, tag="rinv")
                nc.vector.scalar_tensor_tensor(
                    rinv[:], rs0[:], 1e-3, rs1[:], op0=ALU.add, op1=ALU.add,
                )
                nc.vector.reciprocal(rinv[:], rinv[:])
                nc.vector.tensor_scalar_mul(dst, dst, rinv[:])

            def emit_out_dma(b, out_sb, lo, hi):
                # sync ring is idle once inputs are issued; the trigger's
                # semaphore wait blocks nothing there
                nc.sync.dma_start(
                    o_d.ap()[:, b, lo:hi, :], out_sb[:, lo:hi, :]
                )

            ATT1_PAIRS = [(ci, half) for half in range(2) for ci in (0, 2, 4, 6)]

            # ---- DMA schedule (one FIFO ring): s(b0)+weights -> G -> s(b1)
            s0 = phase_load_s(0, first=True)
            nc.sync.dma_start(g_sb[:], g_d.ap())
            s1 = phase_load_s(1)

            # ---- batch 0: fused kq, att1
            ps_kq0 = psKQ.tile([P, N_DIM], f32, tag="kq")
            for m in kq_matmuls(s0, ps_kq0):
                m()
            kq_t0, kq_s0 = emit_evict_kq(ps_kq0)
            att1e0 = att1e_pool.tile([P, NCH, N_DIM], bf16, tag="att1e")
            att1sq0 = att1q_pool.tile([P, NCH, N_DIM], f8, tag="att1q")
            for ci, half in ATT1_PAIRS:
                emit_att1_pair(att1e0, att1sq0, kq_t0, kq_s0, ci, half, 3)

            # ---- att2(b0) with kq/att1 of batch 1 woven into the stream
            out_sb0 = out_pool.tile([P, NCH, IN_DIM], f16, tag="out")
            ps_kq1 = psKQ.tile([P, N_DIM], f32, tag="kq")
            kq_ins = kq_matmuls(s1, ps_kq1)
            nk = len(kq_ins)
            att1e1 = att1e_pool.tile([P, NCH, N_DIM], bf16, tag="att1e")
            att1sq1 = att1q_pool.tile([P, NCH, N_DIM], f8, tag="att1q")
            holder = {}

            for nt in range(NCH):
                phase_att2_group(0, att1sq0, out_sb0, nt)
                if nt == 3:
                    emit_out_dma(0, out_sb0, 0, 4)
                elif nt == 5:
                    emit_out_dma(0, out_sb0, 4, 6)
                if nt == 1:
                    for m in kq_ins[0:nk // 2]:
                        m()
                elif nt == 2:
                    for m in kq_ins[nk // 2:]:
                        m()
                    holder["kq1"] = emit_evict_kq(ps_kq1)
                elif nt == 4:
                    t, s_ = holder["kq1"]
                    for ci, half in ATT1_PAIRS[0:3]:
                        emit_att1_pair(att1e1, att1sq1, t, s_, ci, half, 8)
                elif nt == 5:
                    t, s_ = holder["kq1"]
                    for ci, half in ATT1_PAIRS[3:5]:
                        emit_att1_pair(att1e1, att1sq1, t, s_, ci, half, 8)
                elif nt == 6:
                    t, s_ = holder["kq1"]
                    for ci, half in ATT1_PAIRS[5:7]:
                        emit_att1_pair(att1e1, att1sq1, t, s_, ci, half, 8)
                elif nt == 7:
                    t, s_ = holder["kq1"]
                    for ci, half in ATT1_PAIRS[7:8]:
                        emit_att1_pair(att1e1, att1sq1, t, s_, ci, half, 8)
            emit_out_dma(0, out_sb0, 6, 8)

            out_sb1 = out_pool.tile([P, NCH, IN_DIM], f16, tag="out")
            for nt in range(NCH):
                phase_att2_group(1, att1sq1, out_sb1, nt)
                if nt == 2:
                    emit_out_dma(1, out_sb1, 0, 3)
                elif nt == 5:
                    emit_out_dma(1, out_sb1, 3, 6)
                elif nt == 6:
                    emit_out_dma(1, out_sb1, 6, 7)
            emit_out_dma(1, out_sb1, 7, 8)

    nc.compile()
    return nc


def _get_nc(mode=MODE):
    if mode not in _NC_CACHE:
        _NC_CACHE[mode] = _build_nc(mode)
    return _NC_CACHE[mode]


def _pack(a):
    """[C*P, ...tail] -> [P, C, ...tail]: row c*P+p -> [p][c]."""
    c = a.shape[0] // P
    return np.ascontiguousarray(
        a.reshape(c, P, *a.shape[1:]).swapaxes(0, 1)
    )


def _run(inputs, trace=False, mm_mode=None, tmpdir=None, mode=MODE):
    import ml_dtypes
    from concourse.bass_utils import run_bass_kernel_spmd

    f8 = ml_dtypes.float8_e4m3

    s = np.asarray(inputs["s"], dtype=np.float32)
    g8 = _pack(np.asarray(inputs["Gmat"], np.float32).astype(f8))
    qw = (np.asarray(inputs["Qweight"], np.float32) * 32.0).astype(f8)
    kw = (np.asarray(inputs["Kweight"], np.float32) * 32.0).astype(f8)

    # block-diagonal fused kq weights: [P, NCH, 2, P]
    #   subtile0 (pairs s_n): cols 0-63 = Kw, cols 64-127 = 0
    #   subtile1 (pairs s_T): cols 0-63 = 0,  cols 64-127 = Qw
    wkq = np.zeros((P, NCH, 2, P), f8)
    kw_p = _pack(kw)   # [P, NCH, H]
    qw_p = _pack(qw)
    wkq[:, :, 0, 0:H_DIM] = kw_p
    wkq[:, :, 1, H_DIM:P] = qw_p

    s_c = s.astype(f8)

    nc = _get_nc(mode)
    in_maps = []
    for c in range(N_CORES):
        sl = s_c[:, c * B_LOC:(c + 1) * B_LOC, :]            # [N, B_LOC, I]
        sn = _pack(sl)                                        # [P, NCH, B_LOC, I]
        st = _pack(np.ascontiguousarray(sl.transpose(2, 1, 0)))
        # s[p, b, ch, 0, :] = s_n chunk, s[p, b, ch, 1, :] = s_T chunk
        sb = np.empty((P, B_LOC, NCH, 2, N_DIM), f8)
        sb[:, :, :, 0, :] = sn.transpose(0, 2, 1, 3)
        sb[:, :, :, 1, :] = st.transpose(0, 2, 1, 3)
        in_maps.append({
            "s": sb,
            "gmat": g8,
            "wkq": wkq,
        })
    res = run_bass_kernel_spmd(
        nc, in_maps, list(range(N_CORES)), trace=trace, tmpdir=tmpdir
    )
    outs = []
    for c in range(N_CORES):
        o = res.results[c]["out"]                             # [P, B_LOC, NCH, I]
        outs.append(o.transpose(2, 0, 1, 3).reshape(N_DIM, B_LOC, IN_DIM))
    out = np.concatenate(outs, axis=1).astype(np.float32)
    return out, res


def kernel(**inputs) -> np.ndarray:
    out, _ = _run(inputs, trace=False)
    return out
